# revision 15
# baseline (speedup 1.0000x reference)
"""Trainium2 Bass kernel for nn_CustomRetrieverModel (retrieval_knn).

Late-interaction retriever scoring:
  sim4d = l2n(q_tok) @ l2n(d_tok * punct).T  -> max over doc tokens
  -> valid-weighted mean over query tokens -> avg_sim (B, M)
  logits = shuffle(avg_sim) * shuffle(Wq) * exp(log_inv_t)
  with Wq from L2-normalized CLS vectors: (center - min cand)/2.

Sharding: data-parallel over the M (document) axis. Each of the 8 cores
scores all B=32 queries against M/8 = 8 docs; q_tok/q_cls replicated,
host concatenates the per-core (B, 8) logits and applies the even/odd
column shuffle (a pure output permutation commutes with the elementwise
finale).

Device-side plan (per core):
  - q^T, d^T built via PE transposes (fp32 DMA transpose unsupported).
    The d-side transpose streams diag(mask/||d||) instead of identity,
    fusing punctuation/pad masking + L2 normalization into the
    transpose matmul for free.
  - q is NOT normalized on device: max over doc tokens commutes with the
    positive row scale 1/||q||, which is folded into the weighted-sum
    matmul weights (q_valid/||q||) instead.
  - main matmul: (2048x768) @ (768x2048) in float32r (full-rate fp32 PE
    path), accumulated over 6 K-chunks into PSUM; DVE reduce_max over
    each doc's 256 columns; per-q-chunk weighted-sum matmul accumulates
    the (32, 8) sum_sim directly in PSUM.
  - pad d tokens are zeroed (not -1e-9-masked): only changes the max
    when every real token sims below -1e-9, an O(1e-9) absolute effect.
"""

import sys

for _p in ("/opt/trn_rl_repo",):
    if _p not in sys.path:
        sys.path.append(_p)

import math

import numpy as np

import concourse.bass as bass
import concourse.tile as tile
from concourse import bacc, mybir
import concourse.bass_utils as bass_utils

# ---- problem shape (hardcoded per spec) ----
B, LQ, M, LD, H, L = 32, 64, 64, 256, 768, 3
NCORES = 8
MLOC = M // NCORES          # 8 docs per core
BQ = B * LQ                 # 2048 query rows
DR = MLOC * LD              # 2048 doc-token rows per core
KC = H // 128               # 6 contraction chunks
QT = BQ // 128              # 16 q row tiles
DT = DR // 128              # 16 d row tiles

EPS_NORM = 1e-12
EPS_DIV = 1e-10
LN2 = math.log(2.0)

F32 = mybir.dt.float32
I32 = mybir.dt.int32
U8 = mybir.dt.uint8

# ---- tuning flags ----
MM_DT = mybir.dt.float32r   # main-matmul operand dtype view
DIAG_TRANSPOSE = False      # transpose mode requires a permutation rhs; use
                            # a regular matmul against diag(scale) instead
COPY_ENG = "scalar"         # engine for PSUM->SBUF transpose copies


def _emit(nc, tc, io):
    q_r = io["q_r"].ap()          # (2048, 768) f32   replicated
    d_s = io["d_s"].ap()          # (2048, 768) f32   doc shard rows
    qids = io["qids"].ap()        # (32, 64)   i32    replicated
    dids = io["dids"].ap()        # (2048,)    i32    shard
    dpun = io["dpun"].ap()        # (2048,)    u8     shard
    qcls = io["qcls"].ap()        # (32, 768)  f32    q_cls[-1] replicated
    dcls = io["dcls"].ap()        # (24, 768)  f32    d_cls shard (l*8+m, h)
    logt = io["logt"].ap()        # (32, 1)    f32    log_inv_t replicated
    out = io["out"].ap()          # (32, 8)    f32

    AF = mybir.ActivationFunctionType
    ALU = mybir.AluOpType

    copy_eng = {"scalar": nc.scalar, "vector": nc.vector}[COPY_ENG]


    import contextlib
    ctx = contextlib.ExitStack()
    singles = ctx.enter_context(tc.tile_pool(name="singles", bufs=1))
    smalls = ctx.enter_context(tc.tile_pool(name="smalls", bufs=4))
    clsbig = ctx.enter_context(tc.tile_pool(name="clsbig", bufs=1))

    # ---------- constants & masks ----------
    ident = singles.tile([128, 128], F32)
    nc.vector.memset(ident, 1.0)
    nc.gpsimd.affine_select(
        out=ident, in_=ident, pattern=[[-1, 128]], base=0,
        channel_multiplier=1, compare_op=ALU.is_equal, fill=0.0,
    )

    # q_ids in per-tile layout: row r = c*128+p  ->  (p, c)
    qid_t = singles.tile([128, QT], I32)
    nc.sync.dma_start(qid_t, qids.rearrange("a b -> (a b)").rearrange("(c p) -> p c", p=128))
    qv = singles.tile([128, QT], F32)       # 1.0 where q_ids != 0
    nc.vector.tensor_scalar(qv, qid_t, 0.0, None, op0=ALU.is_equal)
    nc.vector.tensor_scalar(qv, qv, -1.0, 1.0, op0=ALU.mult, op1=ALU.add)

    # n_valid from the natural (32, 64) layout: 64 - sum(q_ids == 0)
    qid_n = smalls.tile([32, 64], I32)
    nc.sync.dma_start(qid_n, qids)
    qv_n = smalls.tile([32, 64], F32)
    nc.vector.tensor_scalar(qv_n, qid_n, 0.0, None, op0=ALU.is_equal)
    nv_eq = smalls.tile([32, 1], F32)
    nc.vector.reduce_sum(nv_eq, qv_n, axis=mybir.AxisListType.X)
    n_valid = smalls.tile([32, 1], F32)     # 64 - sum(eq) + eps
    nc.vector.tensor_scalar(n_valid, nv_eq, -1.0, 64.0 + EPS_DIV, op0=ALU.mult, op1=ALU.add)
    rnv = smalls.tile([32, 1], F32)
    nc.vector.reciprocal(rnv, n_valid)

    # d masks in per-tile layout
    did_t = singles.tile([128, DT], I32)
    nc.sync.dma_start(did_t, dids.rearrange("(c p) -> p c", p=128))
    dpun_t = singles.tile([128, DT], U8)
    nc.sync.dma_start(dpun_t, dpun.rearrange("(c p) -> p c", p=128))
    dmask = singles.tile([128, DT], F32)    # punct & (d_ids != 0)
    nc.vector.tensor_scalar(dmask, did_t, 0.0, None, op0=ALU.is_equal)
    nc.vector.tensor_scalar(dmask, dmask, -1.0, 1.0, op0=ALU.mult, op1=ALU.add)
    dpun_f = singles.tile([128, DT], F32)
    nc.vector.tensor_copy(dpun_f, dpun_t)
    nc.vector.tensor_mul(dmask, dmask, dpun_f)

    lt_t = smalls.tile([32, 1], F32)
    nc.sync.dma_start(lt_t, logt)

    # ---------- rsqrt-with-clamp helper (ACT sqrt + Newton polish) ----------
    def rsqrt_clamped(dst, ss, pool):
        """dst = 1 / max(sqrt(ss), EPS_NORM), elementwise; fp32-accurate."""
        shape = list(ss.shape)
        n0 = pool.tile(shape, F32, tag="rsq_n0")
        nc.scalar.sqrt(n0, ss)
        nc.vector.tensor_scalar_max(n0, n0, 1e-30)
        r0 = pool.tile(shape, F32, tag="rsq_r0")
        nc.vector.reciprocal(r0, n0)
        t = pool.tile(shape, F32, tag="rsq_t")
        nc.vector.tensor_mul(t, ss, r0)
        nc.vector.tensor_add(t, t, n0)
        nc.vector.tensor_scalar(t, t, 0.5, EPS_NORM, op0=ALU.mult, op1=ALU.max)
        nc.vector.reciprocal(dst, t)

    qss = singles.tile([128, QT], F32)
    dss = singles.tile([128, DT], F32)

    # big SBUF transposed operands (written rounded-to-f32r by the copies)
    qT = singles.tile([128, KC, BQ], MM_DT)
    dT = singles.tile([128, KC, DR], MM_DT)

    with tc.tile_pool(name="rows", bufs=10) as rows, \
         tc.tile_pool(name="drs", bufs=6) as drs, \
         tc.tile_pool(name="sqscr", bufs=3) as sqscr, \
         tc.tile_pool(name="ps", bufs=6, space="PSUM") as ps_pool, \
         tc.tile_pool(name="ws", bufs=1, space="PSUM") as ws_pool, \
         tc.tile_pool(name="clsps", bufs=1, space="PSUM") as cls_pool, \
         tc.tile_pool(name="maxs", bufs=6) as maxs_pool:

        # ---------- prep: interleaved q/d groups ----------
        # q groups: DMA 4 row-tiles, sumsq on DVE, 6x raw 512-wide transposes
        # d groups: DMA, sumsq on ACT, row-scale mask/||d|| on ACT, transposes
        for g in range(QT // 4):
            qtiles = []
            for j in range(4):
                c = 4 * g + j
                qt_ = rows.tile([128, H], F32, tag="rowtile")
                nc.sync.dma_start(qt_, q_r[c * 128:(c + 1) * 128, :])
                # sumsq on DVE via bn_stats: ss = n*(var + mean^2)
                stats = smalls.tile([128, 3, 6], F32, tag="bnst")
                for s in range(3):
                    nc.vector.bn_stats(stats[:, s, :], qt_[:, s * 256:(s + 1) * 256])
                mv = smalls.tile([128, 2], F32, tag="bnmv")
                nc.vector.bn_aggr(mv, stats)
                msq = smalls.tile([128, 1], F32, tag="bnmsq")
                nc.vector.tensor_mul(msq, mv[:, 0:1], mv[:, 0:1])
                nc.vector.tensor_add(msq, msq, mv[:, 1:2])
                nc.vector.tensor_scalar_mul(qss[:, c:c + 1], msq, float(H))
                qtiles.append(qt_)
            for k in range(KC):
                tp = ps_pool.tile([128, 512], F32, tag="ps")
                for j in range(4):
                    nc.tensor.transpose(
                        tp[:, j * 128:(j + 1) * 128],
                        qtiles[j][:, k * 128:(k + 1) * 128], ident)
                copy_eng.copy(qT[:, k, g * 512:(g + 1) * 512], tp)

            dtiles = []
            for j in range(4):
                c = 4 * g + j
                dt_ = rows.tile([128, H], F32, tag="rowtile")
                nc.sync.dma_start(dt_, d_s[c * 128:(c + 1) * 128, :])
                scr = sqscr.tile([128, H], F32, tag="sq")
                nc.scalar.activation(scr, dt_, AF.Square, accum_out=dss[:, c:c + 1])
                dtiles.append(dt_)
            # ds = dmask / max(||d||, eps), then scale rows on ACT
            sl = slice(4 * g, 4 * g + 4)
            ds4 = smalls.tile([128, 4], F32, tag="ds4")
            rsqrt_clamped(ds4, dss[:, sl], smalls)
            nc.vector.tensor_mul(ds4, ds4, dmask[:, sl])
            dstiles = []
            for j in range(4):
                dsc = drs.tile([128, H], F32, tag="drow_s")
                nc.scalar.mul(dsc, dtiles[j], ds4[:, j:j + 1])
                dstiles.append(dsc)
            for k in range(KC):
                tp = ps_pool.tile([128, 512], F32, tag="ps")
                for j in range(4):
                    nc.tensor.transpose(
                        tp[:, j * 128:(j + 1) * 128],
                        dstiles[j][:, k * 128:(k + 1) * 128], ident)
                copy_eng.copy(dT[:, k, g * 512:(g + 1) * 512], tp)

        # ---------- CLS weights (Wq) ----------
        qc2 = clsbig.tile([32, H], F32, tag="qc2")
        nc.sync.dma_start(qc2, qcls)
        dcf = clsbig.tile([24, H], F32, tag="dcf")
        nc.sync.dma_start(dcf, dcls)

        qcss = smalls.tile([32, 1], F32, tag="qcss")
        scr1 = clsbig.tile([32, H], F32, tag="clsscr")
        nc.scalar.activation(scr1, qc2, AF.Square, accum_out=qcss)
        dcss = smalls.tile([24, 1], F32, tag="dcss")
        scr2 = clsbig.tile([24, H], F32, tag="clsscr24")
        nc.scalar.activation(scr2, dcf, AF.Square, accum_out=dcss)

        rqc = smalls.tile([32, 1], F32, tag="rqc")
        rsqrt_clamped(rqc, qcss, smalls)
        rdc = smalls.tile([24, 1], F32, tag="rdc")
        rsqrt_clamped(rdc, dcss, smalls)
        nc.scalar.mul(qc2, qc2, rqc)
        nc.scalar.mul(dcf, dcf, rdc)

        qcT = clsbig.tile([128, KC, 32], F32, tag="qcT")
        dcT = clsbig.tile([128, KC, 24], F32, tag="dcT")
        for k in range(KC):
            t1 = ps_pool.tile([128, 32], F32, tag="ps")
            nc.tensor.transpose(t1, qc2[:, k * 128:(k + 1) * 128], ident[0:32, 0:32])
            copy_eng.copy(qcT[:, k, :], t1)
            t2 = ps_pool.tile([128, 24], F32, tag="ps")
            nc.tensor.transpose(t2, dcf[:, k * 128:(k + 1) * 128], ident[0:24, 0:24])
            copy_eng.copy(dcT[:, k, :], t2)

        cls_ps = cls_pool.tile([32, 24], F32)
        for k in range(KC):
            nc.tensor.matmul(cls_ps, qcT[:, k, :], dcT[:, k, :],
                             start=(k == 0), stop=(k == KC - 1))

        cls_sb = smalls.tile([32, 24], F32, tag="cls_sb")
        nc.scalar.copy(cls_sb, cls_ps)
        mind = smalls.tile([32, 8], F32, tag="mind")
        nc.vector.tensor_tensor(mind, cls_sb[:, 0:8], cls_sb[:, 8:16], op=ALU.min)
        wq2 = smalls.tile([32, 8], F32, tag="wq2")  # center - min_doc
        nc.vector.tensor_sub(wq2, cls_sb[:, 16:24], mind)

        # ---------- q norm -> weighted-sum weights W ----------
        rq = smalls.tile([128, QT], F32)
        rsqrt_clamped(rq, qss, smalls)
        wqw = smalls.tile([128, QT], F32)       # q_valid / ||q||
        nc.vector.tensor_mul(wqw, qv, rq)
        W = singles.tile([128, QT, 2 * QT], F32)  # block-diagonal (128, 16, 32)
        nc.vector.memset(W, 0.0)
        for c in range(QT):
            nc.vector.tensor_copy(W[0:64, c, 2 * c:2 * c + 1], wqw[0:64, c:c + 1])
            nc.vector.tensor_copy(W[64:128, c, 2 * c + 1:2 * c + 2], wqw[64:128, c:c + 1])

        # ---------- main matmul + max + weighted sum ----------
        ws_ps = ws_pool.tile([32, MLOC], F32)  # sum_sim accumulator (32, 8)
        for qc in range(QT):
            maxs = maxs_pool.tile([128, MLOC], F32, tag="maxs")
            for cg in range(4):
                ps = ps_pool.tile([128, 512], F32, tag="ps")
                lo = cg * 512
                for k in range(KC):
                    nc.tensor.matmul(ps, qT[:, k, qc * 128:(qc + 1) * 128],
                                     dT[:, k, lo:lo + 512],
                                     start=(k == 0), stop=(k == KC - 1))
                nc.vector.reduce_max(
                    maxs[:, 2 * cg:2 * cg + 2],
                    ps[:].rearrange("p (d l) -> p d l", l=LD),
                    axis=mybir.AxisListType.X)
            nc.tensor.matmul(ws_ps, W[:, qc, :], maxs,
                             start=(qc == 0), stop=(qc == QT - 1))

        # ---------- finale ----------
        it_half = smalls.tile([32, 1], F32, tag="ith")  # exp(log_inv_t)/2
        bln2 = smalls.tile([32, 1], F32, tag="bln2")
        nc.vector.memset(bln2, -LN2)
        nc.scalar.activation(it_half, lt_t, AF.Exp, bias=bln2, scale=1.0)

        avg = smalls.tile([32, 8], F32, tag="avg")
        nc.vector.tensor_scalar(avg, ws_ps, rnv, None, op0=ALU.mult)
        nc.vector.tensor_mul(avg, avg, wq2)
        outt = smalls.tile([32, 8], F32, tag="outt")
        nc.vector.tensor_scalar(outt, avg, it_half, None, op0=ALU.mult)
        nc.sync.dma_start(out, outt)

    ctx.close()


_CACHE = {}


def _build():
    if "nc" in _CACHE:
        return _CACHE["nc"]
    nc = bacc.Bacc("TRN2", target_bir_lowering=False, debug=False,
                   num_devices=NCORES)
    io = {
        "q_r": nc.dram_tensor("q_r", [BQ, H], F32, kind="ExternalInput"),
        "d_s": nc.dram_tensor("d_s", [DR, H], F32, kind="ExternalInput"),
        "qids": nc.dram_tensor("qids", [B, LQ], I32, kind="ExternalInput"),
        "dids": nc.dram_tensor("dids", [DR], I32, kind="ExternalInput"),
        "dpun": nc.dram_tensor("dpun", [DR], U8, kind="ExternalInput"),
        "qcls": nc.dram_tensor("qcls", [B, H], F32, kind="ExternalInput"),
        "dcls": nc.dram_tensor("dcls", [L * MLOC, H], F32, kind="ExternalInput"),
        "logt": nc.dram_tensor("logt", [B, 1], F32, kind="ExternalInput"),
        "out": nc.dram_tensor("out", [B, MLOC], F32, kind="ExternalOutput"),
    }
    with tile.TileContext(nc) as tc:
        _emit(nc, tc, io)
    nc.compile()
    _CACHE["nc"] = nc
    return nc


def make_in_maps(q_tok, d_tok, q_cls, d_cls, log_inv_t, q_ids, d_ids,
                 d_punct_mask):
    q_r = np.ascontiguousarray(np.asarray(q_tok, np.float32).reshape(BQ, H))
    qids = np.ascontiguousarray(np.asarray(q_ids, np.int32))
    qcls = np.ascontiguousarray(np.asarray(q_cls, np.float32)[-1])
    logt = np.full((B, 1), np.float32(np.asarray(log_inv_t)), np.float32)
    d_tok = np.asarray(d_tok, np.float32)
    d_cls = np.asarray(d_cls, np.float32)
    d_ids = np.asarray(d_ids, np.int32)
    d_pun = np.asarray(d_punct_mask).astype(np.uint8)
    in_maps = []
    for c in range(NCORES):
        sl = slice(c * MLOC, (c + 1) * MLOC)
        in_maps.append({
            "q_r": q_r,
            "d_s": np.ascontiguousarray(d_tok[sl].reshape(DR, H)),
            "qids": qids,
            "dids": np.ascontiguousarray(d_ids[sl].reshape(DR)),
            "dpun": np.ascontiguousarray(d_pun[sl].reshape(DR)),
            "qcls": qcls,
            "dcls": np.ascontiguousarray(d_cls[:, sl, :].reshape(L * MLOC, H)),
            "logt": logt,
        })
    return in_maps


_PERM = np.concatenate([np.arange(0, M, 2), np.arange(1, M, 2)])


def kernel(q_tok, d_tok, q_cls, d_cls, log_inv_t, q_ids, d_ids, d_punct_mask,
           **run_kwargs):
    nc = _build()
    in_maps = make_in_maps(q_tok, d_tok, q_cls, d_cls, log_inv_t, q_ids,
                           d_ids, d_punct_mask)
    res = bass_utils.run_bass_kernel_spmd(nc, in_maps,
                                          core_ids=list(range(NCORES)),
                                          **run_kwargs)
    full = np.concatenate([res.results[c]["out"] for c in range(NCORES)],
                          axis=1)
    out = full[:, _PERM]
    if run_kwargs:
        kernel.last_results = res
    return out


# revision 18
# speedup vs baseline: 1.3009x; 1.3009x over previous
"""Trainium2 Bass kernel for nn_CustomRetrieverModel (retrieval_knn).

Late-interaction retriever scoring:
  sim4d = l2n(q_tok) @ l2n(d_tok * punct).T  -> max over doc tokens
  -> valid-weighted mean over query tokens -> avg_sim (B, M)
  logits = shuffle(avg_sim) * shuffle(Wq) * exp(log_inv_t)
  with Wq from L2-normalized CLS vectors: (center - min cand)/2.

Sharding: data-parallel over the M (document) axis. Each of the 8 cores
scores all B=32 queries against M/8 = 8 docs; q_tok/q_cls replicated,
host concatenates the per-core (B, 8) logits and applies the even/odd
column shuffle (a pure output permutation commutes with the elementwise
finale).

Device-side plan (per core):
  - q^T, d^T built via PE transposes (fp32 DMA transpose unsupported).
    The d-side transpose streams diag(mask/||d||) instead of identity,
    fusing punctuation/pad masking + L2 normalization into the
    transpose matmul for free.
  - q is NOT normalized on device: max over doc tokens commutes with the
    positive row scale 1/||q||, which is folded into the weighted-sum
    matmul weights (q_valid/||q||) instead.
  - main matmul: (2048x768) @ (768x2048) in float32r (full-rate fp32 PE
    path), accumulated over 6 K-chunks into PSUM; DVE reduce_max over
    each doc's 256 columns; per-q-chunk weighted-sum matmul accumulates
    the (32, 8) sum_sim directly in PSUM.
  - pad d tokens are zeroed (not -1e-9-masked): only changes the max
    when every real token sims below -1e-9, an O(1e-9) absolute effect.
"""

import sys

for _p in ("/opt/trn_rl_repo",):
    if _p not in sys.path:
        sys.path.append(_p)

import math

import numpy as np

import concourse.bass as bass
import concourse.tile as tile
from concourse import bacc, mybir
import concourse.bass_utils as bass_utils

# ---- problem shape (hardcoded per spec) ----
B, LQ, M, LD, H, L = 32, 64, 64, 256, 768, 3
NCORES = 8
MLOC = M // NCORES          # 8 docs per core
BQ = B * LQ                 # 2048 query rows
DR = MLOC * LD              # 2048 doc-token rows per core
KC = H // 128               # 6 contraction chunks
QT = BQ // 128              # 16 q row tiles
DT = DR // 128              # 16 d row tiles

EPS_NORM = 1e-12
EPS_DIV = 1e-10
LN2 = math.log(2.0)

F32 = mybir.dt.float32
I32 = mybir.dt.int32
U8 = mybir.dt.uint8

# ---- tuning flags ----
MM_DT = mybir.dt.float32r   # main-matmul operand dtype view
DIAG_TRANSPOSE = False      # transpose mode requires a permutation rhs; use
                            # a regular matmul against diag(scale) instead
COPY_ENG = "scalar"         # engine for PSUM->SBUF transpose copies


def _emit(nc, tc, io):
    q_r = io["q_r"].ap()          # (2048, 768) f32   replicated
    d_s = io["d_s"].ap()          # (2048, 768) f32   doc shard rows
    qids = io["qids"].ap()        # (32, 64)   i32    replicated
    qcls = io["qcls"].ap()        # (32, 768)  f32    q_cls[-1] replicated
    dcls = io["dcls"].ap()        # (24, 768)  f32    d_cls shard (l*8+m, h)
    logt = io["logt"].ap()        # (32, 1)    f32    log_inv_t replicated
    out = io["out"].ap()          # (32, 8)    f32

    AF = mybir.ActivationFunctionType
    ALU = mybir.AluOpType

    copy_eng = {"scalar": nc.scalar, "vector": nc.vector}[COPY_ENG]


    import contextlib
    ctx = contextlib.ExitStack()
    singles = ctx.enter_context(tc.tile_pool(name="singles", bufs=1))
    smalls = ctx.enter_context(tc.tile_pool(name="smalls", bufs=4))
    clsbig = ctx.enter_context(tc.tile_pool(name="clsbig", bufs=1))

    # ---------- constants & masks ----------
    ident = singles.tile([128, 128], F32)
    nc.vector.memset(ident, 1.0)
    nc.gpsimd.affine_select(
        out=ident, in_=ident, pattern=[[-1, 128]], base=0,
        channel_multiplier=1, compare_op=ALU.is_equal, fill=0.0,
    )

    # q_ids in per-tile layout (host pre-transposed): tile[p, c] = ids[c*128+p]
    qid_t = singles.tile([128, QT], I32)
    nc.gpsimd.dma_start(qid_t, io["qids_t"].ap())
    qv = singles.tile([128, QT], F32)       # 1.0 where q_ids != 0
    nc.vector.tensor_scalar(qv, qid_t, 0.0, None, op0=ALU.is_equal)
    nc.vector.tensor_scalar(qv, qv, -1.0, 1.0, op0=ALU.mult, op1=ALU.add)

    # n_valid from the natural (32, 64) layout: 64 - sum(q_ids == 0)
    qid_n = smalls.tile([32, 64], I32)
    nc.gpsimd.dma_start(qid_n, qids)
    qv_n = smalls.tile([32, 64], F32)
    nc.vector.tensor_scalar(qv_n, qid_n, 0.0, None, op0=ALU.is_equal)
    nv_eq = smalls.tile([32, 1], F32)
    nc.vector.reduce_sum(nv_eq, qv_n, axis=mybir.AxisListType.X)
    n_valid = smalls.tile([32, 1], F32)     # 64 - sum(eq) + eps
    nc.vector.tensor_scalar(n_valid, nv_eq, -1.0, 64.0 + EPS_DIV, op0=ALU.mult, op1=ALU.add)
    rnv = smalls.tile([32, 1], F32)
    nc.vector.reciprocal(rnv, n_valid)

    # d masks in per-tile layout
    did_t = singles.tile([128, DT], I32)
    nc.gpsimd.dma_start(did_t, io["dids_t"].ap())
    dpun_t = singles.tile([128, DT], U8)
    nc.gpsimd.dma_start(dpun_t, io["dpun_t"].ap())
    dmask = singles.tile([128, DT], F32)    # punct & (d_ids != 0)
    nc.vector.tensor_scalar(dmask, did_t, 0.0, None, op0=ALU.is_equal)
    nc.vector.tensor_scalar(dmask, dmask, -1.0, 1.0, op0=ALU.mult, op1=ALU.add)
    dpun_f = singles.tile([128, DT], F32)
    nc.vector.tensor_copy(dpun_f, dpun_t)
    nc.vector.tensor_mul(dmask, dmask, dpun_f)

    lt_t = smalls.tile([32, 1], F32)
    nc.gpsimd.dma_start(lt_t, logt)

    # ---------- rsqrt-with-clamp helper (ACT sqrt + Newton polish) ----------
    def rsqrt_clamped(dst, ss, pool):
        """dst = 1 / max(sqrt(ss), EPS_NORM), elementwise; fp32-accurate."""
        shape = list(ss.shape)
        n0 = pool.tile(shape, F32, tag="rsq_n0")
        nc.scalar.sqrt(n0, ss)
        nc.vector.tensor_scalar_max(n0, n0, 1e-30)
        r0 = pool.tile(shape, F32, tag="rsq_r0")
        nc.vector.reciprocal(r0, n0)
        t = pool.tile(shape, F32, tag="rsq_t")
        nc.vector.tensor_mul(t, ss, r0)
        nc.vector.tensor_add(t, t, n0)
        nc.vector.tensor_scalar(t, t, 0.5, EPS_NORM, op0=ALU.mult, op1=ALU.max)
        nc.vector.reciprocal(dst, t)

    qss = singles.tile([128, QT], F32)
    dss = singles.tile([128, DT], F32)

    # big SBUF transposed operands (written rounded-to-f32r by the copies)
    qT = singles.tile([128, KC, BQ], MM_DT)
    dT = singles.tile([128, KC, DR], MM_DT)

    with tc.tile_pool(name="rows", bufs=10) as rows, \
         tc.tile_pool(name="drs", bufs=6) as drs, \
         tc.tile_pool(name="sqscr", bufs=3) as sqscr, \
         tc.tile_pool(name="ps", bufs=6, space="PSUM") as ps_pool, \
         tc.tile_pool(name="ws", bufs=1, space="PSUM") as ws_pool, \
         tc.tile_pool(name="clsps", bufs=1, space="PSUM") as cls_pool, \
         tc.tile_pool(name="maxs", bufs=QT) as maxs_pool:

        # ---------- CLS weights (Wq) -- early: doubles as PE warmup ----------
        qc2 = clsbig.tile([32, H], F32, tag="qc2")
        nc.sync.dma_start(qc2, qcls)
        dcf = clsbig.tile([24, H], F32, tag="dcf")
        nc.sync.dma_start(dcf, dcls)

        qcss = smalls.tile([32, 1], F32, tag="qcss")
        scr1 = clsbig.tile([32, H], F32, tag="clsscr")
        nc.scalar.activation(scr1, qc2, AF.Square, accum_out=qcss)
        dcss = smalls.tile([24, 1], F32, tag="dcss")
        scr2 = clsbig.tile([24, H], F32, tag="clsscr24")
        nc.scalar.activation(scr2, dcf, AF.Square, accum_out=dcss)

        rqc = smalls.tile([32, 1], F32, tag="rqc")
        rsqrt_clamped(rqc, qcss, smalls)
        rdc = smalls.tile([24, 1], F32, tag="rdc")
        rsqrt_clamped(rdc, dcss, smalls)
        nc.scalar.mul(qc2, qc2, rqc)
        nc.scalar.mul(dcf, dcf, rdc)

        qcT = clsbig.tile([128, KC, 32], F32, tag="qcT")
        dcT = clsbig.tile([128, KC, 24], F32, tag="dcT")
        for k in range(KC):
            t1 = ps_pool.tile([128, 32], F32, tag="ps")
            nc.tensor.transpose(t1, qc2[:, k * 128:(k + 1) * 128], ident[0:32, 0:32])
            copy_eng.copy(qcT[:, k, :], t1)
            t2 = ps_pool.tile([128, 24], F32, tag="ps")
            nc.tensor.transpose(t2, dcf[:, k * 128:(k + 1) * 128], ident[0:24, 0:24])
            copy_eng.copy(dcT[:, k, :], t2)

        cls_ps = cls_pool.tile([32, 24], F32)
        for k in range(KC):
            nc.tensor.matmul(cls_ps, qcT[:, k, :], dcT[:, k, :],
                             start=(k == 0), stop=(k == KC - 1))

        cls_sb = smalls.tile([32, 24], F32, tag="cls_sb")
        nc.scalar.copy(cls_sb, cls_ps)
        mind = smalls.tile([32, 8], F32, tag="mind")
        nc.vector.tensor_tensor(mind, cls_sb[:, 0:8], cls_sb[:, 8:16], op=ALU.min)
        wq2 = smalls.tile([32, 8], F32, tag="wq2")  # center - min_doc
        nc.vector.tensor_sub(wq2, cls_sb[:, 16:24], mind)

        # ---------- prep groups interleaved with main-matmul blocks ----------
        # emission order == per-engine program order, so main (qc, cg) blocks
        # are emitted as soon as q-group qc//4 and d-group cg exist; they fill
        # the PE while the next group's ACT/DVE/DMA work completes.
        maxs_tiles = [None] * QT

        def q_group(g):
            qtiles = []
            for j in range(4):
                c = 4 * g + j
                qt_ = rows.tile([128, H], F32, tag="rowtile")
                nc.sync.dma_start(qt_, q_r[c * 128:(c + 1) * 128, :])
                # sumsq on DVE via bn_stats: ss = n*(var + mean^2)
                stats = smalls.tile([128, 3, 6], F32, tag="bnst")
                for s in range(3):
                    nc.vector.bn_stats(stats[:, s, :], qt_[:, s * 256:(s + 1) * 256])
                mv = smalls.tile([128, 2], F32, tag="bnmv")
                nc.vector.bn_aggr(mv, stats)
                msq = smalls.tile([128, 1], F32, tag="bnmsq")
                nc.vector.tensor_mul(msq, mv[:, 0:1], mv[:, 0:1])
                nc.vector.tensor_add(msq, msq, mv[:, 1:2])
                nc.vector.tensor_scalar_mul(qss[:, c:c + 1], msq, float(H))
                qtiles.append(qt_)
            for k in range(KC):
                tp = ps_pool.tile([128, 512], F32, tag="ps")
                for j in range(4):
                    nc.tensor.transpose(
                        tp[:, j * 128:(j + 1) * 128],
                        qtiles[j][:, k * 128:(k + 1) * 128], ident)
                copy_eng.copy(qT[:, k, g * 512:(g + 1) * 512], tp)

        def d_group(g):
            dtiles = []
            for j in range(4):
                c = 4 * g + j
                dt_ = rows.tile([128, H], F32, tag="rowtile")
                nc.sync.dma_start(dt_, d_s[c * 128:(c + 1) * 128, :])
                scr = sqscr.tile([128, H], F32, tag="sq")
                nc.scalar.activation(scr, dt_, AF.Square, accum_out=dss[:, c:c + 1])
                dtiles.append(dt_)
            # ds = dmask / max(||d||, eps); row-scale on DVE (2x mode)
            sl = slice(4 * g, 4 * g + 4)
            ds4 = smalls.tile([128, 4], F32, tag="ds4")
            rsqrt_clamped(ds4, dss[:, sl], smalls)
            nc.vector.tensor_mul(ds4, ds4, dmask[:, sl])
            dstiles = []
            for j in range(4):
                dsc = drs.tile([128, H], F32, tag="drow_s")
                nc.vector.tensor_scalar(dsc, dtiles[j], ds4[:, j:j + 1], None,
                                        op0=ALU.mult)
                dstiles.append(dsc)
            for k in range(KC):
                tp = ps_pool.tile([128, 512], F32, tag="ps")
                for j in range(4):
                    nc.tensor.transpose(
                        tp[:, j * 128:(j + 1) * 128],
                        dstiles[j][:, k * 128:(k + 1) * 128], ident)
                copy_eng.copy(dT[:, k, g * 512:(g + 1) * 512], tp)

        def main_block(qc, cg):
            if maxs_tiles[qc] is None:
                maxs_tiles[qc] = maxs_pool.tile([128, MLOC], F32, tag="maxs", name=f"maxs{qc}")
            ps = ps_pool.tile([128, 512], F32, tag="ps")
            lo = cg * 512
            for k in range(KC):
                nc.tensor.matmul(ps, qT[:, k, qc * 128:(qc + 1) * 128],
                                 dT[:, k, lo:lo + 512],
                                 start=(k == 0), stop=(k == KC - 1))
            nc.vector.reduce_max(
                maxs_tiles[qc][:, 2 * cg:2 * cg + 2],
                ps[:].rearrange("p (d l) -> p d l", l=LD),
                axis=mybir.AxisListType.X)

        emitted = set()
        for g in range(4):
            q_group(g)
            d_group(g)
            for qc in range(4 * (g + 1)):
                for cg in range(g + 1):
                    if (qc, cg) not in emitted:
                        emitted.add((qc, cg))
                        main_block(qc, cg)

        # ---------- q norm -> weighted-sum weights W ----------
        rq = smalls.tile([128, QT], F32)
        rsqrt_clamped(rq, qss, smalls)
        wqw = smalls.tile([128, QT], F32)       # q_valid / ||q||
        nc.vector.tensor_mul(wqw, qv, rq)
        W = singles.tile([128, QT, 2 * QT], F32)  # block-diagonal (128, 16, 32)
        nc.vector.memset(W, 0.0)
        for c in range(QT):
            nc.vector.tensor_copy(W[0:64, c, 2 * c:2 * c + 1], wqw[0:64, c:c + 1])
            nc.vector.tensor_copy(W[64:128, c, 2 * c + 1:2 * c + 2], wqw[64:128, c:c + 1])

        # ---------- weighted sums: sum_sim accumulated in one PSUM bank ----
        ws_ps = ws_pool.tile([32, MLOC], F32)
        for qc in range(QT):
            nc.tensor.matmul(ws_ps, W[:, qc, :], maxs_tiles[qc],
                             start=(qc == 0), stop=(qc == QT - 1))

        # ---------- finale ----------
        it_half = smalls.tile([32, 1], F32, tag="ith")  # exp(log_inv_t)/2
        bln2 = smalls.tile([32, 1], F32, tag="bln2")
        nc.vector.memset(bln2, -LN2)
        nc.scalar.activation(it_half, lt_t, AF.Exp, bias=bln2, scale=1.0)

        avg = smalls.tile([32, 8], F32, tag="avg")
        nc.vector.tensor_scalar(avg, ws_ps, rnv, None, op0=ALU.mult)
        nc.vector.tensor_mul(avg, avg, wq2)
        outt = smalls.tile([32, 8], F32, tag="outt")
        nc.vector.tensor_scalar(outt, avg, it_half, None, op0=ALU.mult)
        nc.sync.dma_start(out, outt)

    ctx.close()


_CACHE = {}


def _build():
    if "nc" in _CACHE:
        return _CACHE["nc"]
    nc = bacc.Bacc("TRN2", target_bir_lowering=False, debug=False,
                   num_devices=NCORES)
    io = {
        "q_r": nc.dram_tensor("q_r", [BQ, H], F32, kind="ExternalInput"),
        "d_s": nc.dram_tensor("d_s", [DR, H], F32, kind="ExternalInput"),
        "qids": nc.dram_tensor("qids", [B, LQ], I32, kind="ExternalInput"),
        "qids_t": nc.dram_tensor("qids_t", [128, QT], I32, kind="ExternalInput"),
        "dids_t": nc.dram_tensor("dids_t", [128, DT], I32, kind="ExternalInput"),
        "dpun_t": nc.dram_tensor("dpun_t", [128, DT], U8, kind="ExternalInput"),
        "qcls": nc.dram_tensor("qcls", [B, H], F32, kind="ExternalInput"),
        "dcls": nc.dram_tensor("dcls", [L * MLOC, H], F32, kind="ExternalInput"),
        "logt": nc.dram_tensor("logt", [B, 1], F32, kind="ExternalInput"),
        "out": nc.dram_tensor("out", [B, MLOC], F32, kind="ExternalOutput"),
    }
    with tile.TileContext(nc) as tc:
        _emit(nc, tc, io)
    nc.compile()
    _CACHE["nc"] = nc
    return nc


def make_in_maps(q_tok, d_tok, q_cls, d_cls, log_inv_t, q_ids, d_ids,
                 d_punct_mask):
    q_r = np.ascontiguousarray(np.asarray(q_tok, np.float32).reshape(BQ, H))
    qids = np.ascontiguousarray(np.asarray(q_ids, np.int32))
    qcls = np.ascontiguousarray(np.asarray(q_cls, np.float32)[-1])
    logt = np.full((B, 1), np.float32(np.asarray(log_inv_t)), np.float32)
    qids_t = np.ascontiguousarray(qids.reshape(QT, 128).T)
    d_tok = np.asarray(d_tok, np.float32)
    d_cls = np.asarray(d_cls, np.float32)
    d_ids = np.asarray(d_ids, np.int32)
    d_pun = np.asarray(d_punct_mask).astype(np.uint8)
    in_maps = []
    for c in range(NCORES):
        sl = slice(c * MLOC, (c + 1) * MLOC)
        in_maps.append({
            "q_r": q_r,
            "d_s": np.ascontiguousarray(d_tok[sl].reshape(DR, H)),
            "qids": qids,
            "qids_t": qids_t,
            "dids_t": np.ascontiguousarray(d_ids[sl].reshape(DT, 128).T),
            "dpun_t": np.ascontiguousarray(d_pun[sl].reshape(DT, 128).T),
            "qcls": qcls,
            "dcls": np.ascontiguousarray(d_cls[:, sl, :].reshape(L * MLOC, H)),
            "logt": logt,
        })
    return in_maps


_PERM = np.concatenate([np.arange(0, M, 2), np.arange(1, M, 2)])


def kernel(q_tok, d_tok, q_cls, d_cls, log_inv_t, q_ids, d_ids, d_punct_mask,
           **run_kwargs):
    nc = _build()
    in_maps = make_in_maps(q_tok, d_tok, q_cls, d_cls, log_inv_t, q_ids,
                           d_ids, d_punct_mask)
    res = bass_utils.run_bass_kernel_spmd(nc, in_maps,
                                          core_ids=list(range(NCORES)),
                                          **run_kwargs)
    full = np.concatenate([res.results[c]["out"] for c in range(NCORES)],
                          axis=1)
    out = full[:, _PERM]
    if run_kwargs:
        kernel.last_results = res
    return out
